# revision 4
# baseline (speedup 1.0000x reference)
"""AngularLayer Trainium2 kernel: [500000, 63] -> [500000, 483].

Per row: 21 (x,y) landmarks -> 210 ordered-pair unit direction vectors
(clipped x/y components), appended to the input row.

Sharded batch-parallel over 8 NeuronCores (62500 rows/core), SPMD one graph.
Layout per core: tiles of [125 partitions x R rows], features on the free
axis.  DVE does pair differences / norm-sum / scale / clip, ACT does
squares + rsqrt, DMA streams rows HBM<->SBUF.
"""

import os
from contextlib import ExitStack

import numpy as np

import concourse.bass as bass
import concourse.mybir as mybir
import concourse.tile as tile
from concourse import bacc
from concourse.bass_utils import run_bass_kernel_spmd

F32 = mybir.dt.float32
AF = mybir.ActivationFunctionType
ALU = mybir.AluOpType

N_CORES = 8
B_FULL = 500000
B_SHARD = B_FULL // N_CORES  # 62500
PARTS = 125
ROWS_PER_PART = 10
NLM = 21
NPAIR = 210
IN_C = 63
OUT_C = 483

_RSQRT_MODE = os.environ.get("ANGULAR_RSQRT", "ars")  # "ars" | "lnexp"


def _build_nc(b_shard: int, rows_per_part: int, rsqrt_mode: str) -> bass.Bass:
    R = rows_per_part
    assert b_shard % (PARTS * R) == 0
    n_tiles = b_shard // (PARTS * R)

    nc = bacc.Bacc("TRN2", target_bir_lowering=False, debug=False)
    inp = nc.dram_tensor("tensor", [b_shard, IN_C], F32, kind="ExternalInput")
    outp = nc.dram_tensor("out", [b_shard, OUT_C], F32, kind="ExternalOutput")

    with tile.TileContext(nc) as tc, ExitStack() as ctx:
        opool = ctx.enter_context(tc.tile_pool(name="o", bufs=3))
        vpool = ctx.enter_context(tc.tile_pool(name="vxy", bufs=2))
        sqpool = ctx.enter_context(tc.tile_pool(name="sq", bufs=2))
        npool = ctx.enter_context(tc.tile_pool(name="nsq", bufs=2))
        rpool = ctx.enter_context(tc.tile_pool(name="rs", bufs=2))

        for t in range(n_tiles):
            base = t * PARTS * R
            o = opool.tile([PARTS, R * OUT_C], F32)
            o3 = o[:].rearrange("p (r c) -> p r c", c=OUT_C)

            src = inp[base:base + PARTS * R, :].rearrange("(p r) c -> p r c", p=PARTS)
            nc.sync.dma_start(out=o3[:, :, 0:IN_C], in_=src.opt())

            vxy = vpool.tile([PARTS, R * 2 * NPAIR], F32)
            vxy4 = vxy[:].rearrange("p (r q two) -> p r q two", q=NPAIR, two=2)
            pb = 0
            for i in range(NLM - 1):
                np_i = NLM - 1 - i
                minu = o3[:, :, 3 * (i + 1):IN_C].rearrange(
                    "p r (k three) -> p r k three", three=3)[:, :, :, 0:2]
                subt = o3[:, :, 3 * i:3 * i + 2].unsqueeze(2).broadcast_to(
                    (PARTS, R, np_i, 2))
                nc.vector.tensor_sub(vxy4[:, :, pb:pb + np_i, :], minu, subt)
                pb += np_i

            sq = sqpool.tile([PARTS, R * 2 * NPAIR], F32)
            nc.scalar.activation(sq[:], vxy[:], AF.Square)

            sq4 = sq[:].rearrange("p (r q two) -> p r q two", q=NPAIR, two=2)
            nsq = npool.tile([PARTS, R * NPAIR], F32)
            nsq3 = nsq[:].rearrange("p (r q) -> p r q", q=NPAIR)
            nc.gpsimd.tensor_add(nsq3, sq4[:, :, :, 0], sq4[:, :, :, 1])

            rs = rpool.tile([PARTS, R * NPAIR], F32)
            if rsqrt_mode == "ars":
                nc.scalar.activation(rs[:], nsq[:], AF.Abs_reciprocal_sqrt)
            else:
                nc.scalar.activation(rs[:], nsq[:], AF.Ln)
                nc.scalar.activation(rs[:], rs[:], AF.Exp, scale=-0.5)

            o_tilt4 = o3[:, :, IN_C:OUT_C].rearrange(
                "p r (q two) -> p r q two", two=2)
            rs4 = rs[:].rearrange("p (r q) -> p r q", q=NPAIR).unsqueeze(
                3).broadcast_to((PARTS, R, NPAIR, 2))
            nc.vector.tensor_mul(o_tilt4, vxy4, rs4)

            o_tilt = o3[:, :, IN_C:OUT_C]
            nc.gpsimd.tensor_scalar(o_tilt, o_tilt, 1.0, -1.0, ALU.min, ALU.max)

            dst = outp[base:base + PARTS * R, :].rearrange(
                "(p r) c -> p (r c)", p=PARTS)
            nc.sync.dma_start(out=dst, in_=o[:])

    nc.compile()
    return nc


_NC_CACHE: dict = {}


def _get_nc():
    key = (B_SHARD, ROWS_PER_PART, _RSQRT_MODE)
    if key not in _NC_CACHE:
        _NC_CACHE[key] = _build_nc(B_SHARD, ROWS_PER_PART, _RSQRT_MODE)
    return _NC_CACHE[key]


def kernel(tensor: np.ndarray) -> np.ndarray:
    tensor = np.ascontiguousarray(np.asarray(tensor, dtype=np.float32))
    assert tensor.shape == (B_FULL, IN_C), tensor.shape

    nc = _get_nc()
    in_maps = [
        {"tensor": tensor[c * B_SHARD:(c + 1) * B_SHARD]} for c in range(N_CORES)
    ]
    trace = os.environ.get("ANGULAR_TRACE", "0") == "1"
    res = run_bass_kernel_spmd(
        nc, in_maps, core_ids=list(range(N_CORES)), trace=trace
    )
    if trace:
        kernel.last_exec_time_ns = res.exec_time_ns
        kernel.last_results = res
    out = np.concatenate([res.results[c]["out"] for c in range(N_CORES)], axis=0)
    return out


# revision 6
# speedup vs baseline: 1.1280x; 1.1280x over previous
"""AngularLayer Trainium2 kernel: [500000, 63] -> [500000, 483].

Per row: 21 (x,y) landmarks -> 210 ordered-pair unit direction vectors
(clipped x/y components), appended to the input row.

Sharded batch-parallel over 8 NeuronCores (62500 rows/core), SPMD one graph.
Layout per core: tiles of [125 partitions x R rows], features on the free
axis.  DVE does pair differences / norm-sum / scale / clip, ACT does
squares + rsqrt, DMA streams rows HBM<->SBUF.
"""

import os
from contextlib import ExitStack

import numpy as np

import concourse.bass as bass
import concourse.mybir as mybir
import concourse.tile as tile
from concourse import bacc
from concourse.bass_utils import run_bass_kernel_spmd

F32 = mybir.dt.float32
AF = mybir.ActivationFunctionType
ALU = mybir.AluOpType

N_CORES = 8
B_FULL = 500000
B_SHARD = B_FULL // N_CORES  # 62500
PARTS = 125
ROWS_PER_PART = 25
NLM = 21
NPAIR = 210
IN_C = 63
OUT_C = 483
BF16 = mybir.dt.bfloat16

_RSQRT_MODE = os.environ.get("ANGULAR_RSQRT", "ars")  # "ars" | "lnexp"


def _build_nc(b_shard: int, rows_per_part: int, rsqrt_mode: str) -> bass.Bass:
    R = rows_per_part
    assert b_shard % (PARTS * R) == 0
    n_tiles = b_shard // (PARTS * R)

    nc = bacc.Bacc("TRN2", target_bir_lowering=False, debug=False)
    inp = nc.dram_tensor("tensor", [b_shard, IN_C], F32, kind="ExternalInput")
    outp = nc.dram_tensor("out", [b_shard, OUT_C], F32, kind="ExternalOutput")

    with tile.TileContext(nc) as tc, ExitStack() as ctx:
        opool = ctx.enter_context(tc.tile_pool(name="o", bufs=2))
        vpool = ctx.enter_context(tc.tile_pool(name="vxy", bufs=1))
        sqxp = ctx.enter_context(tc.tile_pool(name="sqx", bufs=1))
        sqyp = ctx.enter_context(tc.tile_pool(name="sqy", bufs=1))
        npool = ctx.enter_context(tc.tile_pool(name="nsq", bufs=1))
        rrpool = ctx.enter_context(tc.tile_pool(name="rr", bufs=1))
        tpool = ctx.enter_context(tc.tile_pool(name="tt", bufs=1))

        for t in range(n_tiles):
            base = t * PARTS * R
            o = opool.tile([PARTS, R * OUT_C], F32)
            o3 = o[:].rearrange("p (r c) -> p r c", c=OUT_C)

            src = inp[base:base + PARTS * R, :].rearrange("(p r) c -> p r c", p=PARTS)
            nc.sync.dma_start(out=o3[:, :, 0:IN_C], in_=src.opt())

            # pair differences: f32 in (strided), bf16 out (unit)  [DVE 1x]
            vxy = vpool.tile([PARTS, R * 2 * NPAIR], BF16)
            vxy4 = vxy[:].rearrange("p (r q two) -> p r q two", q=NPAIR, two=2)
            pb = 0
            for i in range(NLM - 1):
                np_i = NLM - 1 - i
                minu = o3[:, :, 3 * (i + 1):IN_C].rearrange(
                    "p r (k three) -> p r k three", three=3)[:, :, :, 0:2]
                subt = o3[:, :, 3 * i:3 * i + 2].unsqueeze(2).broadcast_to(
                    (PARTS, R, np_i, 2))
                nc.vector.tensor_sub(vxy4[:, :, pb:pb + np_i, :], minu, subt)
                pb += np_i

            # squares, deinterleaved  [ACT]
            sqx = sqxp.tile([PARTS, R * NPAIR], BF16)
            sqy = sqyp.tile([PARTS, R * NPAIR], BF16)
            sqx3 = sqx[:].rearrange("p (r q) -> p r q", q=NPAIR)
            sqy3 = sqy[:].rearrange("p (r q) -> p r q", q=NPAIR)
            nc.scalar.activation(sqx3, vxy4[:, :, :, 0], AF.Square)
            nc.scalar.activation(sqy3, vxy4[:, :, :, 1], AF.Square)

            # nsq = sqx + sqy, all unit bf16  [DVE 2x]
            nsq = npool.tile([PARTS, R * NPAIR], BF16)
            nc.vector.tensor_add(nsq[:], sqx[:], sqy[:])

            # rr = rsqrt(nsq) duplicated into pair-interleaved layout  [ACT x2]
            nsq3 = nsq[:].rearrange("p (r q) -> p r q", q=NPAIR)
            rr = rrpool.tile([PARTS, R * 2 * NPAIR], BF16)
            rr4 = rr[:].rearrange("p (r q two) -> p r q two", q=NPAIR, two=2)
            if rsqrt_mode == "ars":
                nc.scalar.activation(rr4[:, :, :, 0], nsq3, AF.Abs_reciprocal_sqrt)
                nc.scalar.activation(rr4[:, :, :, 1], nsq3, AF.Abs_reciprocal_sqrt)
            else:
                ln = tpool.tile([PARTS, R * 2 * NPAIR], BF16, tag="tt")
                ln3 = ln[:].rearrange("p (r q two) -> p r q two",
                                      q=NPAIR, two=2)[:, :, :, 0]
                nc.scalar.activation(ln3, nsq3, AF.Ln)
                nc.scalar.activation(rr4[:, :, :, 0], ln3, AF.Exp, scale=-0.5)
                nc.scalar.activation(rr4[:, :, :, 1], ln3, AF.Exp, scale=-0.5)

            # tilts = vxy * rr, all flat unit bf16  [DVE 2x]
            tt = tpool.tile([PARTS, R * 2 * NPAIR], BF16, tag="tt")
            nc.vector.tensor_mul(tt[:], vxy[:], rr[:])

            # clip + convert bf16 -> f32 into output cols  [GP]
            o_tilt = o3[:, :, IN_C:OUT_C]
            tt3 = tt[:].rearrange("p (r c) -> p r c", c=2 * NPAIR)
            nc.gpsimd.tensor_scalar(o_tilt, tt3, 1.0, -1.0, ALU.min, ALU.max)

            dst = outp[base:base + PARTS * R, :].rearrange(
                "(p r) c -> p (r c)", p=PARTS)
            nc.sync.dma_start(out=dst, in_=o[:])

    nc.compile()
    return nc


_NC_CACHE: dict = {}


def _get_nc():
    key = (B_SHARD, ROWS_PER_PART, _RSQRT_MODE)
    if key not in _NC_CACHE:
        _NC_CACHE[key] = _build_nc(B_SHARD, ROWS_PER_PART, _RSQRT_MODE)
    return _NC_CACHE[key]


def kernel(tensor: np.ndarray) -> np.ndarray:
    tensor = np.ascontiguousarray(np.asarray(tensor, dtype=np.float32))
    assert tensor.shape == (B_FULL, IN_C), tensor.shape

    nc = _get_nc()
    in_maps = [
        {"tensor": tensor[c * B_SHARD:(c + 1) * B_SHARD]} for c in range(N_CORES)
    ]
    trace = os.environ.get("ANGULAR_TRACE", "0") == "1"
    res = run_bass_kernel_spmd(
        nc, in_maps, core_ids=list(range(N_CORES)), trace=trace
    )
    if trace:
        kernel.last_exec_time_ns = res.exec_time_ns
        kernel.last_results = res
    out = np.concatenate([res.results[c]["out"] for c in range(N_CORES)], axis=0)
    return out
